# revision 1
# baseline (speedup 1.0000x reference)
"""Block-sparse to_dense (scatter-add) on 8 Trainium2 NeuronCores.

Problem: block_values [2048, 64, 64, 8] f32 scatter-added into a dense
[4096, 4096, 8] f32 at 64-aligned positions given by block_indices [2048, 2]
(block row/col in a 64x64 grid). Overlapping blocks sum; out-of-range blocks
drop (indices are block-aligned and H=W=4096=64*64, so partial clipping is
impossible - a block is either fully inside or fully outside).

Strategy (uniform SPMD program, all irregularity in host-prepared data):
  - The dense output is a 64x64 grid of cells; rows are grouped into 32
    "row-pair" slabs of 128 rows. Each core owns 4 slabs (position q=0..3),
    assigned by sorting slabs by block count so padding is minimal and load
    is balanced.
  - Host routes blocks: vals[core] = gathered flattened blocks [M_tot, 32768],
    sel[core] = 0/1 selection matrices [R_tot, 128, 128] mapping slot -> cell.
  - Device, per slab: cells[128, 32768] = sel^T @ vals_rows on the
    TensorEngine. Empty cells get zeros for free; overlapping blocks sum in
    fp32 PSUM. To keep near-fp32 precision at 2x the fp32 PE rate, vals are
    shipped as a bf16 hi/lo pair (x ~= hi + lo, error ~2^-18) and each chunk
    runs two bf16 matmuls accumulating into the same PSUM bank (same total
    input bytes as fp32). Each PSUM -> SBUF copy is split half to the
    Vector and half to the Scalar engine (halves the copy latency at psum
    group boundaries, ~2% measured); input
    DMAs ride the qAct HWDGE ring and output DMAs the qSP ring, and all DMA
    partition counts are multiples of 32 so descriptors spread evenly over
    all 16 SDMA engines (measured ~2x over the naive split).
"""
import numpy as np

N_CORES = 8
B = 64
GRID = 64
KS = 8
H = W = 4096
FLAT = B * B * KS          # 32768 values per block
QS = 4                     # row-pair slabs per core
N_PAIRS = 32

# device loop tiling (chunks of 512 values along FLAT; 64 chunks total)
CH = 512                   # one chunk = one output row-in-block (2 KB fp32)
N_CHUNK = FLAT // CH       # 64
CH_PER_PSUM = 2            # chunks per psum tile  [128, 1024] fp32
CH_PER_STAGE = 16          # chunks per stage-in DMA [m, 8192] 2B (~1.2 MB)
CH_PER_OUT = 8             # chunks per out tile [128, 4096] f32 -> 2 x 1 MB DMAs


# ----------------------------------------------------------------- host prep
def _plan_routing(block_indices):
    idx = np.asarray(block_indices).astype(np.int64)
    r, c = idx[:, 0], idx[:, 1]
    valid = (r >= 0) & (r < GRID) & (c >= 0) & (c < GRID)
    pair = r // 2

    ids_by_pair = [[] for _ in range(N_PAIRS)]
    for n in np.nonzero(valid)[0]:
        ids_by_pair[pair[n]].append(int(n))
    counts = np.array([len(x) for x in ids_by_pair])

    order = np.argsort(-counts, kind="stable")
    pair_of = [[0] * QS for _ in range(N_CORES)]
    ids = [[None] * QS for _ in range(N_CORES)]
    m_q = []
    for q in range(QS):
        grp = order[q * N_CORES:(q + 1) * N_CORES]
        # round up to a multiple of 32: DMA descriptor groups spread over
        # engines in chunks of ceil(round32(parts)/16), so multiples of 32
        # hit all 16 SDMA engines evenly (measured on HW)
        m_q.append(max(32, -(-int(counts[grp].max()) // 32) * 32))
        for core in range(N_CORES):
            pair_of[core][q] = int(grp[core])
            ids[core][q] = ids_by_pair[int(grp[core])]
    rounds = [(m + 127) // 128 for m in m_q]
    row0 = np.concatenate([[0], np.cumsum(m_q)]).astype(int)
    sel_idx = np.concatenate([[0], np.cumsum(rounds)]).astype(int)
    return dict(pair_of=pair_of, ids=ids, m_q=m_q, rounds=rounds, row0=row0,
                sel_idx=sel_idx, M_tot=int(row0[-1]), R_tot=int(sel_idx[-1]),
                r=r, c=c)


def _build_core_inputs(plan, bv_flat, core):
    """vals_hi bf16 + vals_lo fp16 [M_tot, FLAT], sel in both dtypes.

    x ~= bf16(x) + fp16(x - bf16(x)): the bf16 hi bounds the residual to
    2^-9|x| and the fp16 lo keeps 11 more mantissa bits of it, so the
    reconstruction error is ~2^-21|x| - near-fp32 for this data.
    """
    import ml_dtypes
    bf16 = ml_dtypes.bfloat16
    M_tot, R_tot = plan["M_tot"], plan["R_tot"]
    vals_hi = np.zeros((M_tot, FLAT), dtype=bf16)
    vals_lo = np.zeros((M_tot, FLAT), dtype=np.float16)
    sel_h = np.zeros((R_tot, 128, 128), dtype=bf16)
    sel_l = np.zeros((R_tot, 128, 128), dtype=np.float16)
    r_all, c_all = plan["r"], plan["c"]
    for q in range(QS):
        blks = plan["ids"][core][q]
        r0, s0 = plan["row0"][q], plan["sel_idx"][q]
        if blks:
            x = bv_flat[blks]
            hi = x.astype(bf16)
            vals_hi[r0:r0 + len(blks)] = hi
            vals_lo[r0:r0 + len(blks)] = (x - hi.astype(np.float32)).astype(np.float16)
        for slot, n in enumerate(blks):
            col = int(r_all[n] % 2) * 64 + int(c_all[n])
            sel_h[s0 + slot // 128, slot % 128, col] = 1.0
            sel_l[s0 + slot // 128, slot % 128, col] = 1.0
    return vals_hi, vals_lo, sel_h, sel_l


# -------------------------------------------------------------- bass program
_PROGRAM_CACHE = {}


def _build_program(m_q, ch_psum=CH_PER_PSUM, ch_stage=CH_PER_STAGE,
                   ch_out=CH_PER_OUT, stage_bufs=2, out_bufs=4, psum_bufs=4,
                   copy_split=True, copy_mode="vector", ring_mix=False):
    import concourse.mybir as mybir
    from concourse import bacc
    from concourse.tile import TileContext

    m_q = list(m_q)
    rounds = [(m + 127) // 128 for m in m_q]
    row0 = np.concatenate([[0], np.cumsum(m_q)]).astype(int)
    sel_idx = np.concatenate([[0], np.cumsum(rounds)]).astype(int)
    M_tot, R_tot = int(row0[-1]), int(sel_idx[-1])

    # stage tiles for all rounds of a slab are live at once; shrink the
    # staging footprint when heavy index clustering forces multiple
    # 128-row contraction rounds (never happens for uniform indices)
    r_max = max(rounds)
    if r_max > 1:
        if r_max <= 2:
            ch_stage = min(ch_stage, 8)
        elif r_max <= 4:
            ch_stage = min(ch_stage, 4)
        else:
            ch_stage = min(ch_stage, 2)
        if r_max > 8:
            stage_bufs, out_bufs = 1, 2
        ch_psum = min(ch_psum, ch_stage)
    f32 = mybir.dt.float32
    bf16 = mybir.dt.bfloat16
    fp16 = mybir.dt.float16

    nc = bacc.Bacc(
        "TRN2", target_bir_lowering=False, debug=False, num_devices=N_CORES)
    vals_hi = nc.dram_tensor("vals_hi", [M_tot, FLAT], bf16, kind="ExternalInput")
    vals_lo = nc.dram_tensor("vals_lo", [M_tot, FLAT], fp16, kind="ExternalInput")
    sel_h = nc.dram_tensor("sel_h", [R_tot, 128, 128], bf16, kind="ExternalInput")
    sel_l = nc.dram_tensor("sel_l", [R_tot, 128, 128], fp16, kind="ExternalInput")
    out = nc.dram_tensor("out", [512, W, KS], f32, kind="ExternalOutput")

    # out rows = 128*q + 64*half + tt ; cols = 64*c + w ; innermost ks
    # view: [q, half, c, tt, w*ks] so a (q, half) slice iterates (c, tt, wk)
    # in the same order as SBUF [partition=c, free=(tt, wk)]
    out_v = out[:].rearrange(
        "(q half tt) (c w) k -> q half c tt (w k)",
        q=QS, half=2, tt=B, c=GRID,
    )
    srcs = [vals_hi, vals_lo]
    sels = [sel_h, sel_l]
    sdts = [bf16, fp16]

    with TileContext(nc) as tc:
        with (
            tc.tile_pool(name="spool", bufs=2) as s_pool,
            tc.tile_pool(name="stage", bufs=stage_bufs) as stage_pool,
            tc.tile_pool(name="outp", bufs=out_bufs) as out_pool,
            tc.tile_pool(name="psum", bufs=psum_bufs, space="PSUM") as psum_pool,
        ):
            for q in range(QS):
                nr = rounds[q]
                s_tiles = [[], []]  # [hi/lo][round]
                for hl in range(2):
                    for r in range(nr):
                        st = s_pool.tile([128, 128], sdts[hl], tag=f"s{hl}_{r}")
                        nc.scalar.dma_start(out=st[:], in_=sels[hl][sel_idx[q] + r])
                        s_tiles[hl].append(st)
                stage_tiles = [[None] * nr, [None] * nr]  # [hi/lo][round]
                for og in range(N_CHUNK // ch_out):              # out groups
                    outb = out_pool.tile([128, ch_out * CH], f32, tag="outb")
                    for pg in range(ch_out // ch_psum):          # psum groups
                        t0 = og * ch_out + pg * ch_psum
                        if t0 % ch_stage == 0:
                            for hl in range(2):
                                for r in range(nr):
                                    k = min(128, m_q[q] - 128 * r)
                                    stg = stage_pool.tile(
                                        [128, ch_stage * CH], sdts[hl],
                                        tag=f"stg{hl}_{r}")
                                    eng_in = (nc.sync if (ring_mix and hl == 0)
                                              else nc.scalar)
                                    eng_in.dma_start(
                                        out=stg[:k, :],
                                        in_=srcs[hl][
                                            row0[q] + 128 * r: row0[q] + 128 * r + k,
                                            t0 * CH:(t0 + ch_stage) * CH],
                                    )
                                    stage_tiles[hl][r] = stg
                        psum = psum_pool.tile([128, ch_psum * CH], f32, tag="ps")
                        passes = [(hl, r) for hl in range(2) for r in range(nr)]
                        for i in range(ch_psum):
                            off = ((t0 % ch_stage) + i) * CH
                            for pi, (hl, r) in enumerate(passes):
                                k = min(128, m_q[q] - 128 * r)
                                nc.tensor.matmul(
                                    out=psum[:, i * CH:(i + 1) * CH],
                                    lhsT=s_tiles[hl][r][:k, :],
                                    rhs=stage_tiles[hl][r][:k, off:off + CH],
                                    start=(pi == 0),
                                    stop=(pi == len(passes) - 1),
                                )
                        dst = outb[:, pg * ch_psum * CH:(pg + 1) * ch_psum * CH]
                        if copy_split:
                            hw = ch_psum * CH // 2
                            nc.vector.tensor_copy(out=dst[:, :hw], in_=psum[:, :hw])
                            nc.scalar.copy(out=dst[:, hw:], in_=psum[:, hw:])
                        elif copy_mode == "vector" or pg % 2 == 0:
                            nc.vector.tensor_copy(out=dst, in_=psum[:])
                        else:
                            nc.scalar.copy(out=dst, in_=psum[:])
                    for half in range(2):
                        src = outb[64 * half:64 * half + 64, :].rearrange(
                            "p (t wk) -> p t wk", t=ch_out)
                        eng_out = (nc.scalar if (ring_mix and half == 1)
                                   else nc.sync)
                        eng_out.dma_start(
                            out=out_v[q, half, :,
                                      og * ch_out:(og + 1) * ch_out, :],
                            in_=src,
                        )
    nc.compile()
    return nc


# ------------------------------------------------------------------- kernel
def kernel(block_values, block_indices, block_size=None, ks=None, **kw):
    from concourse import bass_utils

    bv = np.ascontiguousarray(np.asarray(block_values), dtype=np.float32)
    assert bv.shape == (2048, B, B, KS), bv.shape
    bv_flat = bv.reshape(-1, FLAT)

    plan = _plan_routing(block_indices)
    key = tuple(plan["m_q"])
    if key not in _PROGRAM_CACHE:
        _PROGRAM_CACHE[key] = _build_program(plan["m_q"])
    nc = _PROGRAM_CACHE[key]

    in_maps = []
    for core in range(N_CORES):
        vh, vl, sh, sl = _build_core_inputs(plan, bv_flat, core)
        in_maps.append({"vals_hi": vh, "vals_lo": vl, "sel_h": sh, "sel_l": sl})

    res = bass_utils.run_bass_kernel_spmd(nc, in_maps, core_ids=list(range(N_CORES)))

    dense = np.zeros((H, W, KS), dtype=np.float32)
    for core in range(N_CORES):
        o = res.results[core]["out"]
        for q in range(QS):
            p = plan["pair_of"][core][q]
            dense[128 * p:128 * p + 128] = o[128 * q:128 * q + 128]
    return dense



# revision 2
# speedup vs baseline: 1.5158x; 1.5158x over previous
"""Block-sparse to_dense (scatter-add) on 8 Trainium2 NeuronCores.

Problem: block_values [2048, 64, 64, 8] f32 scatter-added into a dense
[4096, 4096, 8] f32 at 64-aligned positions given by block_indices
[2048, 2] (block row/col in a 64x64 grid). Overlapping blocks sum;
out-of-range blocks drop. Indices are block-aligned and H=W=4096=64*64,
so a block is either fully inside or fully outside — no partial clipping.

Strategy (correctness gate is rel_err < 2e-2; fp16 in + fp16 out
quantization costs ~5e-4 norm-wise while every output value is still
summed on-device in fp32 PSUM):
  - Each block lands in exactly one 64x64x8 grid *cell* (same block
    row/col). Only ~1583 of the 4096 cells are nonempty (Poisson 0.5):
    the device computes exactly the nonempty cells and the host scatters
    them into the zero-initialized dense array during the unshard step.
    Device write traffic drops ~60% vs writing the full dense tensor.
  - Nonempty cells are packed into N_CORES*G buckets of <=128 cells
    (PSUM partition dim), balanced by block count (LPT greedy), so every
    bucket contracts over ~128 blocks — one full-depth matmul round.
    Per bucket: out[cell, :] = sel^T @ vals on the TensorEngine, where
    vals = the bucket's blocks flattened [m, 32768] fp16 and sel[m, 128]
    is 0/1 routing each block to its cell slot. Overlap summation happens
    in fp32 PSUM; host-side padding slots produce zeros that are ignored.
  - fp16 end-to-end I/O: vals fp16 (rel err 2^-11), fp32 PSUM, fp16 out.
    ~30 MB/core of HBM traffic (16.8 read + 13 write + sel) vs ~96 MB
    for an fp32-precise full-dense variant => DMA roofline ~80 us.
  - DMA tuning (measured): each DMA carries ~2 us of fixed cost and the
    HWDGE rings drain big DMAs near-serially, so transfers are few and
    large: vals stage-in [128, 16K] fp16 (4.2 MB) on the qAct ring
    (scalar engine), out [n, 16K] fp16 (1.6 MB) on the qSP ring (sync),
    double-buffered stage tiles, 4 outb tiles. Vals row counts are
    padded to multiples of 32 so descriptors spread evenly over all 16
    SDMA engines. PSUM->SBUF copies are split half to Vector, half to
    Scalar, converting f32->fp16 on the fly.
"""
import numpy as np

N_CORES = 8
B = 64
GRID = 64
KS = 8
H = W = 4096
FLAT = B * B * KS          # 32768 values per block/cell
CH = 512                   # one matmul chunk (one PSUM bank, 512 f32)
N_CHUNK = FLAT // CH       # 64

CH_PER_PSUM = 2            # chunks per psum tile  [128, 1024] f32
CH_PER_STAGE = 32          # chunks per stage-in DMA [m, 16384] fp16 (~4 MB)
CH_PER_OUT = 16            # chunks per out tile [n, 8192] fp16 (~1.6 MB)


# ----------------------------------------------------------------- host prep
def _plan_routing(block_indices):
    idx = np.asarray(block_indices).astype(np.int64)
    r, c = idx[:, 0], idx[:, 1]
    valid = (r >= 0) & (r < GRID) & (c >= 0) & (c < GRID)
    cell = r * GRID + c

    cells = {}
    for n in np.nonzero(valid)[0]:
        cells.setdefault(int(cell[n]), []).append(int(n))
    ncells = len(cells)
    if ncells == 0:
        return dict(empty=True)

    G = max(1, -(-ncells // (128 * N_CORES)))
    nb = N_CORES * G
    cap = -(-ncells // nb)
    # heaviest cell -> lightest (by blocks) non-full bucket: balances the
    # matmul contraction length while keeping cells/bucket equal (out slots)
    cell_ids = sorted(cells, key=lambda cc: -len(cells[cc]))
    b_cells = [[] for _ in range(nb)]
    b_blocks = [0] * nb
    open_b = list(range(nb))
    for cc in cell_ids:
        i = min(open_b, key=lambda j: b_blocks[j])
        b_cells[i].append(cc)
        b_blocks[i] += len(cells[cc])
        if len(b_cells[i]) >= cap:
            open_b.remove(i)
    # rank buckets by m desc; q gets ranks [q*8, q*8+8) so the per-q max
    # (which sets the uniform SPMD pad) is tight
    order = sorted(range(nb), key=lambda i: -b_blocks[i])
    bucket_of = [[0] * G for _ in range(N_CORES)]
    for q in range(G):
        for k, i in enumerate(order[q * N_CORES:(q + 1) * N_CORES]):
            bucket_of[k][q] = i
    # round up to a multiple of 32: DMA descriptor groups spread over
    # engines in chunks, so multiples of 32 hit all 16 SDMA engines evenly
    m_q = [max(32, -(-max(b_blocks[i] for i in order[q * 8:(q + 1) * 8])
                     // 32) * 32) for q in range(G)]
    n_q = [max(len(b_cells[i]) for i in order[q * 8:(q + 1) * 8])
           for q in range(G)]
    rounds = [(m + 127) // 128 for m in m_q]
    row0 = np.concatenate([[0], np.cumsum(m_q)]).astype(int)
    sel0 = np.concatenate([[0], np.cumsum(rounds)]).astype(int)
    S0 = np.concatenate([[0], np.cumsum(n_q)]).astype(int)
    return dict(empty=False, G=G, cells=cells, b_cells=b_cells,
                bucket_of=bucket_of, m_q=m_q, n_q=n_q, rounds=rounds,
                row0=row0, sel0=sel0, S0=S0, M_tot=int(row0[-1]),
                R_tot=int(sel0[-1]), S_tot=int(S0[-1]))


def _build_core_inputs(plan, bv_flat, core):
    vals = np.zeros((plan["M_tot"], FLAT), dtype=np.float16)
    sel = np.zeros((plan["R_tot"], 128, 128), dtype=np.float16)
    for q in range(plan["G"]):
        bkt = plan["b_cells"][plan["bucket_of"][core][q]]
        blks, slots = [], []
        for slot, cc in enumerate(bkt):
            ids = plan["cells"][cc]
            blks += ids
            slots += [slot] * len(ids)
        if not blks:
            continue
        r0, s0 = plan["row0"][q], plan["sel0"][q]
        vals[r0:r0 + len(blks)] = bv_flat[blks]  # f32 -> fp16 cast
        for i, slot in enumerate(slots):
            sel[s0 + i // 128, i % 128, slot] = 1.0
    return {"vals": vals, "sel": sel}


# -------------------------------------------------------------- bass program
_PROGRAM_CACHE = {}


def _build_program(n_q, m_q, ch_psum=CH_PER_PSUM, ch_stage=CH_PER_STAGE,
                   ch_out=CH_PER_OUT, stage_bufs=2, out_bufs=4, psum_bufs=4,
                   copy_mode="split", mm_wide=False):
    import concourse.mybir as mybir
    from concourse import bacc
    from concourse.tile import TileContext

    G = len(m_q)
    rounds = [(m + 127) // 128 for m in m_q]
    row0 = np.concatenate([[0], np.cumsum(m_q)]).astype(int)
    sel0 = np.concatenate([[0], np.cumsum(rounds)]).astype(int)
    S0 = np.concatenate([[0], np.cumsum(n_q)]).astype(int)
    M_tot, R_tot, S_tot = int(row0[-1]), int(sel0[-1]), int(S0[-1])

    # stage tiles for all rounds of a bucket are live at once; shrink the
    # staging footprint when heavy index clustering forces many 128-row
    # contraction rounds (never happens for uniform indices)
    r_max = max(rounds)
    if r_max > 1:
        if r_max <= 2:
            ch_stage = min(ch_stage, 8)
        elif r_max <= 4:
            ch_stage = min(ch_stage, 4)
        else:
            ch_stage = min(ch_stage, 2)
        if r_max > 8:
            stage_bufs, out_bufs = 1, 2
        ch_psum = min(ch_psum, ch_stage)
        ch_out = min(ch_out, 8)
    f32 = mybir.dt.float32
    fp16 = mybir.dt.float16

    nc = bacc.Bacc(
        "TRN2", target_bir_lowering=False, debug=False, num_devices=N_CORES)
    vals = nc.dram_tensor("vals", [M_tot, FLAT], fp16, kind="ExternalInput")
    sel = nc.dram_tensor("sel", [R_tot, 128, 128], fp16, kind="ExternalInput")
    out = nc.dram_tensor("out", [S_tot, FLAT], fp16, kind="ExternalOutput")

    with TileContext(nc) as tc:
        with (
            tc.tile_pool(name="spool", bufs=2) as s_pool,
            tc.tile_pool(name="stage", bufs=stage_bufs) as stage_pool,
            tc.tile_pool(name="outp", bufs=out_bufs) as out_pool,
            tc.tile_pool(name="psum", bufs=psum_bufs, space="PSUM") as psum_pool,
        ):
            for q in range(G):
                nr, n = rounds[q], n_q[q]
                if n == 0:
                    continue
                s_tiles = []
                for rr in range(nr):
                    st = s_pool.tile([128, 128], fp16, tag=f"s_{rr}")
                    # sync ring: idle at startup, keeps the scalar queue
                    # free to issue the first vals stage DMA immediately
                    nc.sync.dma_start(out=st[:], in_=sel[sel0[q] + rr])
                    s_tiles.append(st)
                stage_tiles = [None] * nr
                stage_c0 = 0
                for og in range(N_CHUNK // ch_out):              # out groups
                    outb = out_pool.tile([128, ch_out * CH], fp16, tag="outb")
                    for pg in range(ch_out // ch_psum):          # psum groups
                        t0 = og * ch_out + pg * ch_psum
                        if t0 % ch_stage == 0:
                            stage_c0 = t0
                            for rr in range(nr):
                                k = min(128, m_q[q] - 128 * rr)
                                stg = stage_pool.tile(
                                    [128, ch_stage * CH], fp16, tag=f"stg{rr}")
                                nc.scalar.dma_start(
                                    out=stg[:k, :],
                                    in_=vals[
                                        row0[q] + 128 * rr:
                                        row0[q] + 128 * rr + k,
                                        t0 * CH:(t0 + ch_stage) * CH],
                                )
                                stage_tiles[rr] = stg
                        psum = psum_pool.tile([128, ch_psum * CH], f32,
                                              tag="ps")
                        mmw = ch_psum * CH if mm_wide else CH
                        for i in range(ch_psum * CH // mmw):
                            off = (t0 - stage_c0) * CH + i * mmw
                            for ri in range(nr):
                                k = min(128, m_q[q] - 128 * ri)
                                nc.tensor.matmul(
                                    out=psum[:n, i * mmw:(i + 1) * mmw],
                                    lhsT=s_tiles[ri][:k, :n],
                                    rhs=stage_tiles[ri][:k, off:off + mmw],
                                    start=(ri == 0),
                                    stop=(ri == nr - 1),
                                )
                        dst = outb[:n, pg * ch_psum * CH:(pg + 1) * ch_psum * CH]
                        if copy_mode == "split":
                            hw = ch_psum * CH // 2
                            nc.vector.tensor_copy(out=dst[:, :hw],
                                                  in_=psum[:n, :hw])
                            nc.scalar.copy(out=dst[:, hw:], in_=psum[:n, hw:])
                        else:
                            nc.vector.tensor_copy(out=dst, in_=psum[:n, :])
                    nc.sync.dma_start(
                        out=out[S0[q]:S0[q] + n,
                                og * ch_out * CH:(og + 1) * ch_out * CH],
                        in_=outb[:n, :],
                    )
    nc.compile()
    return nc


# ------------------------------------------------------------------- kernel
def kernel(block_values, block_indices, block_size=None, ks=None, **kw):
    from concourse import bass_utils

    bv = np.ascontiguousarray(np.asarray(block_values), dtype=np.float32)
    assert bv.shape == (2048, B, B, KS), bv.shape
    bv_flat = bv.reshape(-1, FLAT)

    plan = _plan_routing(block_indices)
    if plan.get("empty"):
        return np.zeros((H, W, KS), dtype=np.float32)
    key = (tuple(plan["n_q"]), tuple(plan["m_q"]))
    if key not in _PROGRAM_CACHE:
        _PROGRAM_CACHE[key] = _build_program(plan["n_q"], plan["m_q"])
    nc = _PROGRAM_CACHE[key]

    in_maps = [_build_core_inputs(plan, bv_flat, core)
               for core in range(N_CORES)]
    res = bass_utils.run_bass_kernel_spmd(nc, in_maps,
                                          core_ids=list(range(N_CORES)))

    dense = np.zeros((H, W, KS), dtype=np.float32)
    for core in range(N_CORES):
        o = np.asarray(res.results[core]["out"], dtype=np.float32)
        for q in range(plan["G"]):
            s0 = plan["S0"][q]
            for slot, cc in enumerate(
                    plan["b_cells"][plan["bucket_of"][core][q]]):
                rr, cl = divmod(cc, GRID)
                dense[rr * B:(rr + 1) * B, cl * B:(cl + 1) * B, :] = (
                    o[s0 + slot].reshape(B, B, KS))
    return dense
